# revision 16
# baseline (speedup 1.0000x reference)
"""ViT block kernel for Trainium2, data-parallel over batch across 8 cores.

Per-core program (sequence 1024, dim 768, 12 heads, mlp 3072), transposed
[feature, seq] layout on device end-to-end:

  LN1   : column sums via ones-matmul + rank-1 (w x -mu/D) broadcast matmul
  QKV   : weights stationary as M-slabs; V first, then per head-pair m-slab
          so attention pipelines with QKV and PE stays HAM-warm
  attn  : per head pair (2 heads share a 128-row q/k tile):
          - dots^T   = K @ Q^T, K=64 row-packed (tiles T0/T8), 2-bank PSUM
          - exp      = one ACT op per [128, 1024] tile (scale folded)
          - attn@V   = M=64 col-packed (2 heads in one [128,512] PSUM tile)
          - sums     = ones[128,64] stationary -> 64 replicated sum rows per
                       head, same packing; gives the broadcast for free
          - divide   = one DVE reciprocal + one DVE multiply per chunk
  Wo    : + residual accumulated in place into the fp32 x^T tiles
  LN2   : same as LN1
  FFN   : relu(x@W1+b1)@W2 + b2, weights streamed as M-slabs
  out   : transposed output, un-transposed on host

Matmul operands bf16 (fp32 PSUM accumulation); residual stream, softmax
sums and reciprocals, LN stats in fp32.  Host pre-transposes x, pre-casts
weights to bf16, and re-transposes the output.
"""

import numpy as np
import ml_dtypes

import concourse.bass as bass
from concourse import bacc
import concourse.mybir as mybir
import concourse.tile as tile
from concourse.bass import ts, ds
from concourse.bass_utils import run_bass_kernel_spmd

F32 = mybir.dt.float32
BF16 = mybir.dt.bfloat16
AF = mybir.ActivationFunctionType
ALU = mybir.AluOpType

B = 8          # batch == number of cores
N = 1024       # sequence length
D = 768        # model dim
H = 12         # heads
DH = 64        # head dim
F = 3072       # mlp dim
P = 128        # partitions
NT = N // P    # 8 seq tiles
DT = D // P    # 6 dim tiles
FT = F // P    # 24 mlp tiles
NCH = 512      # psum free-dim chunk
NC = N // NCH  # 2 chunks
HP = H // 2    # 6 head pairs
SCALE = DH ** -0.5

MM_SB = BF16
MM_NP = ml_dtypes.bfloat16


def _patch_act_tables():
    """Put the ln+exp table set first so the act-table-load pass picks one
    set for every activation in this kernel (Exp/Ln/Identity/Copy all live
    in natural_log_exp_and_others) instead of thrashing 2.7us reloads."""
    import concourse.hw_specs as _hws
    orig = _hws.get_activation_tables

    def pinned(arch):
        tabs = orig(arch)
        key = "natural_log_exp_and_others"
        if key not in tabs:
            return tabs
        # Preserve dict order (act_func_set_id is positional); empty every
        # other set so the load pass can only choose `key` for our funcs.
        return {k: (v if k == key else set()) for k, v in tabs.items()}

    bacc.get_activation_tables = pinned


def build_program():
    _patch_act_tables()
    nc = bacc.Bacc("TRN2", target_bir_lowering=False)

    xt = nc.dram_tensor("xt", [D, N], F32, kind="ExternalInput").ap()
    wq = nc.dram_tensor("wq", [D, D], MM_SB, kind="ExternalInput").ap()
    wk = nc.dram_tensor("wk", [D, D], MM_SB, kind="ExternalInput").ap()
    wv = nc.dram_tensor("wv", [D, D], MM_SB, kind="ExternalInput").ap()
    wo = nc.dram_tensor("wo", [D, D], MM_SB, kind="ExternalInput").ap()
    w1 = nc.dram_tensor("w1", [D, F], MM_SB, kind="ExternalInput").ap()
    w2 = nc.dram_tensor("w2", [F, D], MM_SB, kind="ExternalInput").ap()
    bo = nc.dram_tensor("bo", [D], F32, kind="ExternalInput").ap()
    b1 = nc.dram_tensor("b1", [F], F32, kind="ExternalInput").ap()
    b2 = nc.dram_tensor("b2", [D], F32, kind="ExternalInput").ap()
    ln1w = nc.dram_tensor("ln1w", [D], F32, kind="ExternalInput").ap()
    ln1b = nc.dram_tensor("ln1b", [D], F32, kind="ExternalInput").ap()
    ln2w = nc.dram_tensor("ln2w", [D], F32, kind="ExternalInput").ap()
    ln2b = nc.dram_tensor("ln2b", [D], F32, kind="ExternalInput").ap()
    outt = nc.dram_tensor("outt", [D, N], F32, kind="ExternalOutput").ap()

    with tile.TileContext(nc) as tc:
        _emit(nc, tc, xt, wq, wk, wv, wo, w1, w2, bo, b1, b2,
              ln1w, ln1b, ln2w, ln2b, outt)
    nc.compile()
    return nc


def _tiles(pool, n, shape, dt, tag):
    return [
        pool.tile(shape, dt, tag=f"{tag}{i}", name=f"{tag}{i}") for i in range(n)
    ]


def _emit(nc, tc, xt, wq, wk, wv, wo, w1, w2, bo, b1, b2,
          ln1w, ln1b, ln2w, ln2b, outt):
    with (
        tc.tile_pool(name="consts", bufs=1) as consts,
        tc.tile_pool(name="rows", bufs=2) as rows,
        tc.tile_pool(name="stage", bufs=4) as stage,
        tc.tile_pool(name="resid", bufs=1) as resid_pool,
    ):
        ones_full = consts.tile([P, P], MM_SB, tag="onesfull")
        nc.gpsimd.memset(ones_full[:], 1.0)

        def col_load(vec, nt, tag):
            t = consts.tile([P, nt], F32, tag=tag)
            nc.sync.dma_start(t[:], vec.rearrange("(t p) -> p t", p=P))
            return t

        ln1w_c = col_load(ln1w, DT, "ln1wc")
        ln1b_c = col_load(ln1b, DT, "ln1bc")
        ln2w_c = col_load(ln2w, DT, "ln2wc")
        ln2b_c = col_load(ln2b, DT, "ln2bc")
        bo_c = col_load(bo, DT, "boc")
        b2_c = col_load(b2, DT, "b2c")
        b1_c = col_load(b1, FT, "b1c")

        def row_load(vec, tag):
            st = consts.tile([1, D], F32, tag=tag + "f")
            nc.sync.dma_start(st[:], vec[None, :])
            t = consts.tile([P, D], MM_SB, tag=tag)
            nc.gpsimd.memset(t[:], 0.0)
            nc.vector.tensor_copy(t[0:1, :], st[:])
            return t

        ln1w_r = row_load(ln1w, "ln1wr")
        ln2w_r = row_load(ln2w, "ln2wr")

        # fp32 residual stream, updated in place phase to phase
        xts = _tiles(resid_pool, DT, [P, N], F32, "xt")
        for j in range(DT):
            nc.sync.dma_start(xts[j][:], xt[ts(j, P), :])

        def layernorm(lnps, src_tiles, src_mm_tiles, w_col, b_col, w_row,
                      out_tiles):
            """out = w * (src - mean_over_dim(src)) + b (transposed layout).

            All matmuls full 128x128 mode: sums via all-ones stationary
            (replicated rows), broadcast via w-in-row-0 stationary against a
            negmu tile with zeroed rows 1..127.
            """
            negmu = rows.tile([P, N], MM_SB, tag="negmu")
            nc.gpsimd.memset(negmu[:], 0.0)
            for ch in range(NC):
                sps = lnps.tile([P, NCH], F32, tag="lns", name="lnsums")
                for k in range(DT):
                    nc.tensor.matmul(
                        sps[:], ones_full[:], src_mm_tiles[k][:, ts(ch, NCH)],
                        start=(k == 0), stop=(k == DT - 1))
                nc.scalar.activation(negmu[0:1, ts(ch, NCH)], sps[0:1, :],
                                     AF.Copy, scale=-1.0 / D)
            for j in range(DT):
                for ch in range(NC):
                    bps = lnps.tile([P, NCH], F32, tag="lnb", name="lnbcast")
                    nc.tensor.matmul(bps[:], w_row[:, ts(j, P)],
                                     negmu[:, ts(ch, NCH)],
                                     start=True, stop=True)
                    t1 = stage.tile([P, NCH], F32, tag="st")
                    nc.scalar.activation(
                        t1[:], src_tiles[j][:, ts(ch, NCH)], AF.Identity,
                        bias=b_col[:, j:j + 1], scale=w_col[:, j:j + 1])
                    nc.vector.tensor_tensor(
                        out=out_tiles[j][:, ts(ch, NCH)], in0=t1[:],
                        in1=bps[:], op=ALU.add)

        # ============ LN1 + QKV + attention (pipelined per head pair) ====
        with (
            tc.tile_pool(name="ln1", bufs=1) as ln1_pool,
            tc.tile_pool(name="wqk", bufs=2) as wqk_pool,
            tc.tile_pool(name="wvp", bufs=1) as wv_pool,
            tc.tile_pool(name="qkv", bufs=1) as qkv_pool,
            tc.tile_pool(name="aot", bufs=1) as aot_pool,
            tc.tile_pool(name="exp", bufs=1) as exp_pool,
        ):
            hbf = _tiles(ln1_pool, DT, [P, N], MM_SB, "h")
            with (
                tc.tile_pool(name="xbfp", bufs=1) as xbf_pool,
                tc.tile_pool(name="lnps1", bufs=1, space="PSUM") as lnps1,
            ):
                xbf = _tiles(xbf_pool, DT, [P, N], MM_SB, "xb")
                for j in range(DT):
                    nc.vector.tensor_copy(xbf[j][:], xts[j][:])
                layernorm(lnps1, xts, xbf, ln1w_c, ln1b_c, ln1w_r, hbf)

            wv_sb = _tiles(wv_pool, DT, [P, D], MM_SB, "wv")
            for m in range(DT):
                nc.sync.dma_start(wv_sb[m][:], wv[ts(m, P), :])

            vbf = _tiles(qkv_pool, NT, [P, D], MM_SB, "v")
            aot = _tiles(aot_pool, DT, [P, N], MM_SB, "ao")

            with (
                tc.tile_pool(name="qkps", bufs=2, space="PSUM") as qkps,
                tc.tile_pool(name="dotps", bufs=2, space="PSUM") as dotps,
                tc.tile_pool(name="avps", bufs=1, space="PSUM") as avps,
            ):
                # ---- V for all heads (enables per-pair attention) ----
                for i in range(NT):
                    for c0, cw in ((0, NCH), (NCH, D - NCH)):
                        vps = qkps.tile([P, NCH], F32, tag="qk", name="vps")
                        for k in range(DT):
                            nc.tensor.matmul(
                                vps[:, :cw], hbf[k][:, ts(i, P)],
                                wv_sb[k][:, ds(c0, cw)],
                                start=(k == 0), stop=(k == DT - 1))
                        nc.vector.tensor_copy(vbf[i][:, ds(c0, cw)],
                                              vps[:, :cw])

                def attnv(t, eta, etb):
                    """attn@V + softmax division for head pair t."""
                    for ch in range(NC):
                        for head, r0, r1, et in ((0, 0, DH, eta),
                                                 (1, DH, P, etb)):
                            av = avps.tile([P, NCH], F32, tag="av",
                                           name="avps")
                            for ki in range(NT):
                                nc.tensor.matmul(
                                    av[:], vbf[ki][:, ts(t, P)],
                                    et[ki][:, ts(ch, NCH)],
                                    start=(ki == 0), stop=(ki == NT - 1))
                            sps = avps.tile([P, NCH], F32, tag="sm",
                                            name="smps")
                            for ki in range(NT):
                                nc.tensor.matmul(
                                    sps[:], ones_full[:],
                                    et[ki][:, ts(ch, NCH)],
                                    start=(ki == 0), stop=(ki == NT - 1))
                            # 1/s = exp(-ln(s)) on ACT: off the DVE, and
                            # both functions live in one activation table set
                            lns = stage.tile([P, NCH], F32, tag="st")
                            nc.scalar.activation(lns[r0:r1, :], sps[r0:r1, :],
                                                 AF.Ln)
                            rec = stage.tile([P, NCH], F32, tag="st")
                            nc.scalar.activation(rec[r0:r1, :], lns[r0:r1, :],
                                                 AF.Exp, scale=-1.0)
                            nc.vector.tensor_tensor(
                                out=aot[t][r0:r1, ts(ch, NCH)],
                                in0=av[r0:r1, :], in1=rec[r0:r1, :],
                                op=ALU.mult)

                # ---- per pair: q/k proj -> dots+exp; attnv lags one pair
                prev = None
                for t in range(HP):
                    wq_sb = wqk_pool.tile([P, DT, P], MM_SB, tag="wq",
                                          name=f"wq{t}")
                    nc.sync.dma_start(
                        wq_sb[:],
                        wq[:, ts(t, P)].rearrange("(t p) m -> p t m", p=P))
                    wk_sb = wqk_pool.tile([P, DT, P], MM_SB, tag="wk",
                                          name=f"wk{t}")
                    nc.sync.dma_start(
                        wk_sb[:],
                        wk[:, ts(t, P)].rearrange("(t p) m -> p t m", p=P))

                    qbt = qkv_pool.tile([P, N], MM_SB, tag=f"q{t % 2}",
                                        name=f"q{t}")
                    kza = qkv_pool.tile([P, N], MM_SB, tag=f"kza{t % 2}",
                                        name=f"kza{t}")
                    kzb = qkv_pool.tile([P, N], MM_SB, tag=f"kzb{t % 2}",
                                        name=f"kzb{t}")
                    nc.gpsimd.memset(kza[DH:P, :], 0.0)
                    nc.gpsimd.memset(kzb[0:DH, :], 0.0)
                    for ch in range(NC):
                        qps = qkps.tile([P, NCH], F32, tag="qk", name="qps")
                        for k in range(DT):
                            nc.tensor.matmul(
                                qps[:], wq_sb[:, k, :],
                                hbf[k][:, ts(ch, NCH)],
                                start=(k == 0), stop=(k == DT - 1))
                        nc.vector.tensor_copy(qbt[:, ts(ch, NCH)], qps[:])
                        kps = qkps.tile([P, NCH], F32, tag="qk", name="kps")
                        for k in range(DT):
                            nc.tensor.matmul(
                                kps[:], wk_sb[:, k, :],
                                hbf[k][:, ts(ch, NCH)],
                                start=(k == 0), stop=(k == DT - 1))
                        nc.vector.tensor_copy(kza[0:DH, ts(ch, NCH)],
                                              kps[0:DH, :])
                        nc.vector.tensor_copy(kzb[DH:P, ts(ch, NCH)],
                                              kps[DH:P, :])

                    # dots (zero-padded K=128, full mode) + exp
                    eta = [exp_pool.tile([P, N], MM_SB, tag="et", bufs=24,
                                         name=f"ea{t}_{i}")
                           for i in range(NT)]
                    etb = [exp_pool.tile([P, N], MM_SB, tag="et", bufs=24,
                                         name=f"eb{t}_{i}")
                           for i in range(NT)]
                    for mi in range(NT):
                        da = dotps.tile([P, N], F32, tag="dot", name="dpsa")
                        db = dotps.tile([P, N], F32, tag="dot", name="dpsb")
                        for ch in range(NC):
                            nc.tensor.matmul(
                                da[:, ts(ch, NCH)], kza[:, ts(mi, P)],
                                qbt[:, ts(ch, NCH)], start=True, stop=True)
                            nc.tensor.matmul(
                                db[:, ts(ch, NCH)], kzb[:, ts(mi, P)],
                                qbt[:, ts(ch, NCH)], start=True, stop=True)
                        nc.scalar.activation(eta[mi][:], da[:], AF.Exp,
                                             scale=SCALE)
                        nc.scalar.activation(etb[mi][:], db[:], AF.Exp,
                                             scale=SCALE)

                    if prev is not None:
                        attnv(*prev)
                    prev = (t, eta, etb)
                attnv(*prev)

                # ---- Wo + residual ----
                for m in range(DT):
                    wo_sb = wqk_pool.tile([P, DT, P], MM_SB, tag="wq",
                                          name=f"wo{m}")
                    nc.sync.dma_start(
                        wo_sb[:],
                        wo[:, ts(m, P)].rearrange("(t p) m -> p t m", p=P))
                    pss = [qkps.tile([P, NCH], F32, tag="qk", name="wops")
                           for _ in range(NC)]
                    for k in range(DT):
                        for ch in range(NC):
                            nc.tensor.matmul(
                                pss[ch][:], wo_sb[:, k, :],
                                aot[k][:, ts(ch, NCH)],
                                start=(k == 0), stop=(k == DT - 1))
                    for ch in range(NC):
                        nc.vector.scalar_tensor_tensor(
                            out=xts[m][:, ts(ch, NCH)], in0=pss[ch][:],
                            scalar=bo_c[:, m:m + 1],
                            in1=xts[m][:, ts(ch, NCH)],
                            op0=ALU.add, op1=ALU.add)

        # ============ LN2 + FFN ============
        with (
            tc.tile_pool(name="ln2", bufs=1) as ln2_pool,
            tc.tile_pool(name="wff", bufs=1) as wff_pool,
            tc.tile_pool(name="ff1", bufs=1) as ff1_pool,
            tc.tile_pool(name="mm", bufs=6, space="PSUM") as mmps,
        ):
            # FFN weight slab DMAs issued first: overlap with LN2 compute
            w1_sb = _tiles(wff_pool, FT, [P, DT, P], MM_SB, "w1")
            for mf in range(FT):
                nc.sync.dma_start(
                    w1_sb[mf][:],
                    w1[:, ts(mf, P)].rearrange("(t p) m -> p t m", p=P))
            w2_sb = _tiles(wff_pool, DT, [P, FT, P], MM_SB, "w2")
            for m in range(DT):
                nc.sync.dma_start(
                    w2_sb[m][:],
                    w2[:, ts(m, P)].rearrange("(t p) m -> p t m", p=P))

            h2bf = _tiles(ln2_pool, DT, [P, N], MM_SB, "h2")
            with (
                tc.tile_pool(name="xmbfp", bufs=1) as xmbf_pool,
                tc.tile_pool(name="lnps2", bufs=1, space="PSUM") as lnps2,
            ):
                xmbf = _tiles(xmbf_pool, DT, [P, N], MM_SB, "xmb")
                for j in range(DT):
                    nc.vector.tensor_copy(xmbf[j][:], xts[j][:])
                layernorm(lnps2, xts, xmbf, ln2w_c, ln2b_c, ln2w_r, h2bf)

            ff1 = ff1_pool.tile([P, FT, NCH], MM_SB, tag="ff1", name="ff1")
            for ch in range(NC):
                for mf in range(FT):
                    ps = mmps.tile([P, NCH], F32, tag="mm", name="f1ps")
                    for k in range(DT):
                        nc.tensor.matmul(
                            ps[:], w1_sb[mf][:, k, :], h2bf[k][:, ts(ch, NCH)],
                            start=(k == 0), stop=(k == DT - 1))
                    nc.vector.tensor_scalar(
                        out=ff1[:, mf, :], in0=ps[:],
                        scalar1=b1_c[:, mf:mf + 1], scalar2=0.0,
                        op0=ALU.add, op1=ALU.max)
                for m in range(DT):
                    ps = mmps.tile([P, NCH], F32, tag="mm", name="f2ps")
                    for kf in range(FT):
                        nc.tensor.matmul(
                            ps[:], w2_sb[m][:, kf, :], ff1[:, kf, :],
                            start=(kf == 0), stop=(kf == FT - 1))
                    ot = stage.tile([P, NCH], F32, tag="st")
                    nc.vector.scalar_tensor_tensor(
                        out=ot[:], in0=ps[:], scalar=b2_c[:, m:m + 1],
                        in1=xts[m][:, ts(ch, NCH)],
                        op0=ALU.add, op1=ALU.add)
                    nc.sync.dma_start(outt[ts(m, P), ts(ch, NCH)], ot[:])


_CACHED = None


def _get_program():
    global _CACHED
    if _CACHED is None:
        _CACHED = build_program()
    return _CACHED


def prepare_in_maps(inputs):
    x = np.asarray(inputs["x"], dtype=np.float32)
    wcast = lambda a: np.ascontiguousarray(np.asarray(a, np.float32)).astype(MM_NP)
    f32c = lambda a: np.ascontiguousarray(np.asarray(a, np.float32))
    shared = {
        "wq": wcast(inputs["Wq"]), "wk": wcast(inputs["Wk"]),
        "wv": wcast(inputs["Wv"]), "wo": wcast(inputs["Wo"]),
        "w1": wcast(inputs["W1"]), "w2": wcast(inputs["W2"]),
        "bo": f32c(inputs["bo"]), "b1": f32c(inputs["b1"]),
        "b2": f32c(inputs["b2"]),
        "ln1w": f32c(inputs["ln1_w"]), "ln1b": f32c(inputs["ln1_b"]),
        "ln2w": f32c(inputs["ln2_w"]), "ln2b": f32c(inputs["ln2_b"]),
    }
    in_maps = []
    for i in range(B):
        m = dict(shared)
        m["xt"] = np.ascontiguousarray(x[i].T)  # [D, N]
        in_maps.append(m)
    return in_maps


def kernel(**inputs):
    nc = _get_program()
    in_maps = prepare_in_maps(inputs)
    res = run_bass_kernel_spmd(nc, in_maps, list(range(B)))
    out = np.stack([np.ascontiguousarray(r["outt"].T) for r in res.results])
    return out.astype(np.float32)


# revision 17
# speedup vs baseline: 1.2295x; 1.2295x over previous
"""ViT block kernel for Trainium2, data-parallel over batch across 8 cores.

Per-core program (sequence 1024, dim 768, 12 heads, mlp 3072), transposed
[feature, seq] layout on device end-to-end:

  LN1   : column sums via ones-matmul + rank-1 (w x -mu/D) broadcast matmul
  QKV   : weights stationary as M-slabs; V first, then per head-pair m-slab
          so attention pipelines with QKV and PE stays HAM-warm
  attn  : per head pair (2 heads share a 128-row q/k tile):
          - dots^T   = K @ Q^T, K=64 row-packed (tiles T0/T8), 2-bank PSUM
          - exp      = one ACT op per [128, 1024] tile (scale folded)
          - attn@V   = M=64 col-packed (2 heads in one [128,512] PSUM tile)
          - sums     = ones[128,64] stationary -> 64 replicated sum rows per
                       head, same packing; gives the broadcast for free
          - divide   = one DVE reciprocal + one DVE multiply per chunk
  Wo    : + residual accumulated in place into the fp32 x^T tiles
  LN2   : same as LN1
  FFN   : relu(x@W1+b1)@W2 + b2, weights streamed as M-slabs
  out   : transposed output, un-transposed on host

Matmul operands bf16 (fp32 PSUM accumulation); residual stream, softmax
sums and reciprocals, LN stats in fp32.  Host pre-transposes x, pre-casts
weights to bf16, and re-transposes the output.
"""

import numpy as np
import ml_dtypes

import concourse.bass as bass
from concourse import bacc
import concourse.mybir as mybir
import concourse.tile as tile
from concourse.bass import ts, ds
from concourse.bass_utils import run_bass_kernel_spmd

F32 = mybir.dt.float32
BF16 = mybir.dt.bfloat16
AF = mybir.ActivationFunctionType
ALU = mybir.AluOpType

B = 8          # batch == number of cores
N = 1024       # sequence length
D = 768        # model dim
H = 12         # heads
DH = 64        # head dim
F = 3072       # mlp dim
P = 128        # partitions
NT = N // P    # 8 seq tiles
DT = D // P    # 6 dim tiles
FT = F // P    # 24 mlp tiles
NCH = 512      # psum free-dim chunk
NC = N // NCH  # 2 chunks
HP = H // 2    # 6 head pairs
SCALE = DH ** -0.5

MM_SB = BF16
MM_NP = ml_dtypes.bfloat16


def _patch_act_tables():
    """Put the ln+exp table set first so the act-table-load pass picks one
    set for every activation in this kernel (Exp/Ln/Identity/Copy all live
    in natural_log_exp_and_others) instead of thrashing 2.7us reloads."""
    import concourse.hw_specs as _hws
    orig = _hws.get_activation_tables

    def pinned(arch):
        tabs = orig(arch)
        key = "natural_log_exp_and_others"
        if key not in tabs:
            return tabs
        # Preserve dict order (act_func_set_id is positional); empty every
        # other set so the load pass can only choose `key` for our funcs.
        return {k: (v if k == key else set()) for k, v in tabs.items()}

    bacc.get_activation_tables = pinned


def build_program():
    _patch_act_tables()
    nc = bacc.Bacc("TRN2", target_bir_lowering=False)

    xt = nc.dram_tensor("xt", [D, N], F32, kind="ExternalInput").ap()
    wq = nc.dram_tensor("wq", [D, D], MM_SB, kind="ExternalInput").ap()
    wk = nc.dram_tensor("wk", [D, D], MM_SB, kind="ExternalInput").ap()
    wv = nc.dram_tensor("wv", [D, D], MM_SB, kind="ExternalInput").ap()
    wo = nc.dram_tensor("wo", [D, D], MM_SB, kind="ExternalInput").ap()
    w1 = nc.dram_tensor("w1", [D, F], MM_SB, kind="ExternalInput").ap()
    w2 = nc.dram_tensor("w2", [F, D], MM_SB, kind="ExternalInput").ap()
    bo = nc.dram_tensor("bo", [D], F32, kind="ExternalInput").ap()
    b1 = nc.dram_tensor("b1", [F], F32, kind="ExternalInput").ap()
    b2 = nc.dram_tensor("b2", [D], F32, kind="ExternalInput").ap()
    ln1w = nc.dram_tensor("ln1w", [D], F32, kind="ExternalInput").ap()
    ln1b = nc.dram_tensor("ln1b", [D], F32, kind="ExternalInput").ap()
    ln2w = nc.dram_tensor("ln2w", [D], F32, kind="ExternalInput").ap()
    ln2b = nc.dram_tensor("ln2b", [D], F32, kind="ExternalInput").ap()
    outt = nc.dram_tensor("outt", [D, N], F32, kind="ExternalOutput").ap()

    with tile.TileContext(nc) as tc:
        _emit(nc, tc, xt, wq, wk, wv, wo, w1, w2, bo, b1, b2,
              ln1w, ln1b, ln2w, ln2b, outt)
    nc.compile()
    return nc


def _tiles(pool, n, shape, dt, tag):
    return [
        pool.tile(shape, dt, tag=f"{tag}{i}", name=f"{tag}{i}") for i in range(n)
    ]


def _emit(nc, tc, xt, wq, wk, wv, wo, w1, w2, bo, b1, b2,
          ln1w, ln1b, ln2w, ln2b, outt):
    with (
        tc.tile_pool(name="consts", bufs=1) as consts,
        tc.tile_pool(name="rows", bufs=2) as rows,
        tc.tile_pool(name="stage", bufs=4) as stage,
        tc.tile_pool(name="resid", bufs=1) as resid_pool,
    ):
        ones_full = consts.tile([P, P], MM_SB, tag="onesfull")
        nc.gpsimd.memset(ones_full[:], 1.0)

        def col_load(vec, nt, tag):
            t = consts.tile([P, nt], F32, tag=tag)
            nc.sync.dma_start(t[:], vec.rearrange("(t p) -> p t", p=P))
            return t

        ln1w_c = col_load(ln1w, DT, "ln1wc")
        ln1b_c = col_load(ln1b, DT, "ln1bc")
        ln2w_c = col_load(ln2w, DT, "ln2wc")
        ln2b_c = col_load(ln2b, DT, "ln2bc")
        bo_c = col_load(bo, DT, "boc")
        b2_c = col_load(b2, DT, "b2c")
        b1_c = col_load(b1, FT, "b1c")

        def row_load(vec, tag):
            st = consts.tile([1, D], F32, tag=tag + "f")
            nc.sync.dma_start(st[:], vec[None, :])
            t = consts.tile([P, D], MM_SB, tag=tag)
            nc.gpsimd.memset(t[:], 0.0)
            nc.vector.tensor_copy(t[0:1, :], st[:])
            return t

        ln1w_r = row_load(ln1w, "ln1wr")
        ln2w_r = row_load(ln2w, "ln2wr")

        # fp32 residual stream, updated in place phase to phase
        xts = _tiles(resid_pool, DT, [P, N], F32, "xt")
        for j in range(DT):
            nc.sync.dma_start(xts[j][:], xt[ts(j, P), :])

        def layernorm(lnps, src_tiles, src_mm_tiles, w_col, b_col, w_row,
                      out_tiles):
            """out = w * (src - mean_over_dim(src)) + b (transposed layout).

            All matmuls full 128x128 mode: sums via all-ones stationary
            (replicated rows), broadcast via w-in-row-0 stationary against a
            negmu tile with zeroed rows 1..127.
            """
            negmu = rows.tile([P, N], MM_SB, tag="negmu")
            nc.gpsimd.memset(negmu[:], 0.0)
            for ch in range(NC):
                sps = lnps.tile([P, NCH], F32, tag="lns", name="lnsums")
                for k in range(DT):
                    nc.tensor.matmul(
                        sps[:], ones_full[:], src_mm_tiles[k][:, ts(ch, NCH)],
                        start=(k == 0), stop=(k == DT - 1))
                nc.scalar.activation(negmu[0:1, ts(ch, NCH)], sps[0:1, :],
                                     AF.Copy, scale=-1.0 / D)
            for j in range(DT):
                for ch in range(NC):
                    bps = lnps.tile([P, NCH], F32, tag="lnb", name="lnbcast")
                    nc.tensor.matmul(bps[:], w_row[:, ts(j, P)],
                                     negmu[:, ts(ch, NCH)],
                                     start=True, stop=True)
                    t1 = stage.tile([P, NCH], F32, tag="st")
                    nc.vector.tensor_scalar(
                        out=t1[:], in0=src_tiles[j][:, ts(ch, NCH)],
                        scalar1=w_col[:, j:j + 1], scalar2=b_col[:, j:j + 1],
                        op0=ALU.mult, op1=ALU.add)
                    nc.vector.tensor_tensor(
                        out=out_tiles[j][:, ts(ch, NCH)], in0=t1[:],
                        in1=bps[:], op=ALU.add)

        # ============ LN1 + QKV + attention (pipelined per head pair) ====
        with (
            tc.tile_pool(name="ln1", bufs=1) as ln1_pool,
            tc.tile_pool(name="wqk", bufs=2) as wqk_pool,
            tc.tile_pool(name="wvp", bufs=1) as wv_pool,
            tc.tile_pool(name="qkv", bufs=1) as qkv_pool,
            tc.tile_pool(name="aot", bufs=1) as aot_pool,
            tc.tile_pool(name="exp", bufs=1) as exp_pool,
        ):
            hbf = _tiles(ln1_pool, DT, [P, N], MM_SB, "h")
            with (
                tc.tile_pool(name="xbfp", bufs=1) as xbf_pool,
                tc.tile_pool(name="lnps1", bufs=1, space="PSUM") as lnps1,
            ):
                xbf = _tiles(xbf_pool, DT, [P, N], MM_SB, "xb")
                for j in range(DT):
                    nc.vector.tensor_copy(xbf[j][:], xts[j][:])
                layernorm(lnps1, xts, xbf, ln1w_c, ln1b_c, ln1w_r, hbf)

            wv_sb = _tiles(wv_pool, DT, [P, D], MM_SB, "wv")
            for m in range(DT):
                nc.sync.dma_start(wv_sb[m][:], wv[ts(m, P), :])

            vbf = _tiles(qkv_pool, NT, [P, H * P], MM_SB, "v")
            aot = _tiles(aot_pool, DT, [P, N], MM_SB, "ao")

            with (
                tc.tile_pool(name="qkps", bufs=2, space="PSUM") as qkps,
                tc.tile_pool(name="dotps", bufs=2, space="PSUM") as dotps,
                tc.tile_pool(name="avps", bufs=2, space="PSUM") as avps,
            ):
                # ---- V for all heads, augmented layout ----
                # head 2t   -> vbf cols [256t,    256t+64) = v, then 64 ones
                # head 2t+1 -> vbf cols [256t+192, 256t+256) = v, ones before
                # so out2 rows and replicated-sum rows alternate alignment.
                for i in range(NT):
                    for t6 in range(HP):
                        nc.gpsimd.memset(vbf[i][:, ds(t6 * 2 * P + DH, P)], 1.0)
                    for c0, cw in ((0, NCH), (NCH, D - NCH)):
                        vps = qkps.tile([P, NCH], F32, tag="qk", name="vps")
                        for k in range(DT):
                            nc.tensor.matmul(
                                vps[:, :cw], hbf[k][:, ts(i, P)],
                                wv_sb[k][:, ds(c0, cw)],
                                start=(k == 0), stop=(k == DT - 1))
                        np_ = cw // P  # head pairs in this chunk
                        src = vps[:, :cw].rearrange("p (t x) -> p t x", x=P)
                        dst = vbf[i][:, ds(c0 * 2, np_ * 2 * P)].rearrange(
                            "p (t x) -> p t x", x=2 * P)
                        nc.vector.tensor_copy(dst[:, :, 0:DH],
                                              src[:, :, 0:DH])
                        nc.vector.tensor_copy(dst[:, :, 3 * DH:4 * DH],
                                              src[:, :, DH:2 * DH])

                def attnv(t, eta, etb):
                    """attn@V for pair t; [v|ones] aug gives sums in the
                    other 64 psum rows; 1/s = exp(-ln(s)) on ACT; DMA shifts
                    the reciprocal rows into alignment with out2."""
                    for ch in range(NC):
                        for head, et in ((0, eta), (1, etb)):
                            # even head: out2 rows 0:64, sums rows 64:128
                            o0, o1 = (0, DH) if head == 0 else (DH, P)
                            s0, s1 = (DH, P) if head == 0 else (0, DH)
                            av = avps.tile([P, NCH], F32, tag="av",
                                           name="avps")
                            for ki in range(NT):
                                nc.tensor.matmul(
                                    av[:], vbf[ki][:, ds((2 * t + head) * P, P)],
                                    et[ki][:, ts(ch, NCH)],
                                    start=(ki == 0), stop=(ki == NT - 1))
                            lns = stage.tile([P, NCH], F32, tag="st")
                            nc.scalar.activation(lns[s0:s1, :], av[s0:s1, :],
                                                 AF.Ln)
                            rec = stage.tile([P, NCH], F32, tag="st")
                            nc.scalar.activation(rec[s0:s1, :], lns[s0:s1, :],
                                                 AF.Exp, scale=-1.0)
                            reca = stage.tile([P, NCH], F32, tag="st")
                            nc.sync.dma_start(reca[o0:o1, :], rec[s0:s1, :])
                            nc.vector.tensor_tensor(
                                out=aot[t][o0:o1, ts(ch, NCH)],
                                in0=av[o0:o1, :], in1=reca[o0:o1, :],
                                op=ALU.mult)

                # ---- per pair: q/k proj -> dots+exp; attnv lags one pair
                prev = None
                for t in range(HP):
                    wq_sb = wqk_pool.tile([P, DT, P], MM_SB, tag="wq",
                                          name=f"wq{t}")
                    nc.sync.dma_start(
                        wq_sb[:],
                        wq[:, ts(t, P)].rearrange("(t p) m -> p t m", p=P))
                    wk_sb = wqk_pool.tile([P, DT, P], MM_SB, tag="wk",
                                          name=f"wk{t}")
                    nc.sync.dma_start(
                        wk_sb[:],
                        wk[:, ts(t, P)].rearrange("(t p) m -> p t m", p=P))

                    qbt = qkv_pool.tile([P, N], MM_SB, tag=f"q{t % 2}",
                                        name=f"q{t}")
                    kza = qkv_pool.tile([P, N], MM_SB, tag=f"kza{t % 2}",
                                        name=f"kza{t}")
                    kzb = qkv_pool.tile([P, N], MM_SB, tag=f"kzb{t % 2}",
                                        name=f"kzb{t}")
                    nc.gpsimd.memset(kza[DH:P, :], 0.0)
                    nc.gpsimd.memset(kzb[0:DH, :], 0.0)
                    for ch in range(NC):
                        qps = qkps.tile([P, NCH], F32, tag="qk", name="qps")
                        for k in range(DT):
                            nc.tensor.matmul(
                                qps[:], wq_sb[:, k, :],
                                hbf[k][:, ts(ch, NCH)],
                                start=(k == 0), stop=(k == DT - 1))
                        nc.vector.tensor_copy(qbt[:, ts(ch, NCH)], qps[:])
                        kps = qkps.tile([P, NCH], F32, tag="qk", name="kps")
                        for k in range(DT):
                            nc.tensor.matmul(
                                kps[:], wk_sb[:, k, :],
                                hbf[k][:, ts(ch, NCH)],
                                start=(k == 0), stop=(k == DT - 1))
                        nc.vector.tensor_copy(kza[0:DH, ts(ch, NCH)],
                                              kps[0:DH, :])
                        nc.vector.tensor_copy(kzb[DH:P, ts(ch, NCH)],
                                              kps[DH:P, :])

                    # dots (zero-padded K=128, full mode) + exp
                    eta = [exp_pool.tile([P, N], MM_SB, tag="et", bufs=24,
                                         name=f"ea{t}_{i}")
                           for i in range(NT)]
                    etb = [exp_pool.tile([P, N], MM_SB, tag="et", bufs=24,
                                         name=f"eb{t}_{i}")
                           for i in range(NT)]
                    for mi in range(NT):
                        da = dotps.tile([P, N], F32, tag="dot", name="dpsa")
                        db = dotps.tile([P, N], F32, tag="dot", name="dpsb")
                        for ch in range(NC):
                            nc.tensor.matmul(
                                da[:, ts(ch, NCH)], kza[:, ts(mi, P)],
                                qbt[:, ts(ch, NCH)], start=True, stop=True)
                            nc.tensor.matmul(
                                db[:, ts(ch, NCH)], kzb[:, ts(mi, P)],
                                qbt[:, ts(ch, NCH)], start=True, stop=True)
                        nc.scalar.activation(eta[mi][:], da[:], AF.Exp,
                                             scale=SCALE)
                        nc.scalar.activation(etb[mi][:], db[:], AF.Exp,
                                             scale=SCALE)

                    if prev is not None:
                        attnv(*prev)
                    prev = (t, eta, etb)
                attnv(*prev)

                # ---- Wo + residual ----
                for m in range(DT):
                    wo_sb = wqk_pool.tile([P, DT, P], MM_SB, tag="wq",
                                          name=f"wo{m}")
                    nc.sync.dma_start(
                        wo_sb[:],
                        wo[:, ts(m, P)].rearrange("(t p) m -> p t m", p=P))
                    pss = [avps.tile([P, NCH], F32, tag="av", name="wops")
                           for _ in range(NC)]
                    for k in range(DT):
                        for ch in range(NC):
                            nc.tensor.matmul(
                                pss[ch][:], wo_sb[:, k, :],
                                aot[k][:, ts(ch, NCH)],
                                start=(k == 0), stop=(k == DT - 1))
                    for ch in range(NC):
                        nc.vector.scalar_tensor_tensor(
                            out=xts[m][:, ts(ch, NCH)], in0=pss[ch][:],
                            scalar=bo_c[:, m:m + 1],
                            in1=xts[m][:, ts(ch, NCH)],
                            op0=ALU.add, op1=ALU.add)

        # ============ LN2 + FFN ============
        with (
            tc.tile_pool(name="ln2", bufs=1) as ln2_pool,
            tc.tile_pool(name="wff", bufs=1) as wff_pool,
            tc.tile_pool(name="ff1", bufs=1) as ff1_pool,
            tc.tile_pool(name="mm", bufs=6, space="PSUM") as mmps,
        ):
            # FFN weight slab DMAs issued first: overlap with LN2 compute
            w1_sb = _tiles(wff_pool, FT, [P, DT, P], MM_SB, "w1")
            for mf in range(FT):
                nc.sync.dma_start(
                    w1_sb[mf][:],
                    w1[:, ts(mf, P)].rearrange("(t p) m -> p t m", p=P))
            w2_sb = _tiles(wff_pool, DT, [P, FT, P], MM_SB, "w2")
            for m in range(DT):
                nc.sync.dma_start(
                    w2_sb[m][:],
                    w2[:, ts(m, P)].rearrange("(t p) m -> p t m", p=P))

            h2bf = _tiles(ln2_pool, DT, [P, N], MM_SB, "h2")
            with (
                tc.tile_pool(name="xmbfp", bufs=1) as xmbf_pool,
                tc.tile_pool(name="lnps2", bufs=1, space="PSUM") as lnps2,
            ):
                xmbf = _tiles(xmbf_pool, DT, [P, N], MM_SB, "xmb")
                for j in range(DT):
                    nc.vector.tensor_copy(xmbf[j][:], xts[j][:])
                layernorm(lnps2, xts, xmbf, ln2w_c, ln2b_c, ln2w_r, h2bf)

            ff1 = ff1_pool.tile([P, FT, NCH], MM_SB, tag="ff1", name="ff1")
            for ch in range(NC):
                for mf in range(FT):
                    ps = mmps.tile([P, NCH], F32, tag="mm", name="f1ps")
                    for k in range(DT):
                        nc.tensor.matmul(
                            ps[:], w1_sb[mf][:, k, :], h2bf[k][:, ts(ch, NCH)],
                            start=(k == 0), stop=(k == DT - 1))
                    nc.vector.tensor_scalar(
                        out=ff1[:, mf, :], in0=ps[:],
                        scalar1=b1_c[:, mf:mf + 1], scalar2=0.0,
                        op0=ALU.add, op1=ALU.max)
                for m in range(DT):
                    ps = mmps.tile([P, NCH], F32, tag="mm", name="f2ps")
                    for kf in range(FT):
                        nc.tensor.matmul(
                            ps[:], w2_sb[m][:, kf, :], ff1[:, kf, :],
                            start=(kf == 0), stop=(kf == FT - 1))
                    ot = stage.tile([P, NCH], F32, tag="st")
                    nc.vector.scalar_tensor_tensor(
                        out=ot[:], in0=ps[:], scalar=b2_c[:, m:m + 1],
                        in1=xts[m][:, ts(ch, NCH)],
                        op0=ALU.add, op1=ALU.add)
                    nc.sync.dma_start(outt[ts(m, P), ts(ch, NCH)], ot[:])


_CACHED = None


def _get_program():
    global _CACHED
    if _CACHED is None:
        _CACHED = build_program()
    return _CACHED


def prepare_in_maps(inputs):
    x = np.asarray(inputs["x"], dtype=np.float32)
    wcast = lambda a: np.ascontiguousarray(np.asarray(a, np.float32)).astype(MM_NP)
    f32c = lambda a: np.ascontiguousarray(np.asarray(a, np.float32))
    shared = {
        "wq": wcast(inputs["Wq"]), "wk": wcast(inputs["Wk"]),
        "wv": wcast(inputs["Wv"]), "wo": wcast(inputs["Wo"]),
        "w1": wcast(inputs["W1"]), "w2": wcast(inputs["W2"]),
        "bo": f32c(inputs["bo"]), "b1": f32c(inputs["b1"]),
        "b2": f32c(inputs["b2"]),
        "ln1w": f32c(inputs["ln1_w"]), "ln1b": f32c(inputs["ln1_b"]),
        "ln2w": f32c(inputs["ln2_w"]), "ln2b": f32c(inputs["ln2_b"]),
    }
    in_maps = []
    for i in range(B):
        m = dict(shared)
        m["xt"] = np.ascontiguousarray(x[i].T)  # [D, N]
        in_maps.append(m)
    return in_maps


def kernel(**inputs):
    nc = _get_program()
    in_maps = prepare_in_maps(inputs)
    res = run_bass_kernel_spmd(nc, in_maps, list(range(B)))
    out = np.stack([np.ascontiguousarray(r["outt"].T) for r in res.results])
    return out.astype(np.float32)


# revision 19
# speedup vs baseline: 1.2604x; 1.0252x over previous
"""ViT block kernel for Trainium2, data-parallel over batch across 8 cores.

Per-core program (sequence 1024, dim 768, 12 heads, mlp 3072), transposed
[feature, seq] layout on device end-to-end:

  LN1   : column sums via ones-matmul + rank-1 (w x -mu/D) broadcast matmul
  QKV   : weights stationary as M-slabs; V first, then per head-pair m-slab
          so attention pipelines with QKV and PE stays HAM-warm
  attn  : per head pair (2 heads share a 128-row q/k tile):
          - dots^T   = K @ Q^T, K=64 row-packed (tiles T0/T8), 2-bank PSUM
          - exp      = one ACT op per [128, 1024] tile (scale folded)
          - attn@V   = M=64 col-packed (2 heads in one [128,512] PSUM tile)
          - sums     = ones[128,64] stationary -> 64 replicated sum rows per
                       head, same packing; gives the broadcast for free
          - divide   = one DVE reciprocal + one DVE multiply per chunk
  Wo    : + residual accumulated in place into the fp32 x^T tiles
  LN2   : same as LN1
  FFN   : relu(x@W1+b1)@W2 + b2, weights streamed as M-slabs
  out   : transposed output, un-transposed on host

Matmul operands bf16 (fp32 PSUM accumulation); residual stream, softmax
sums and reciprocals, LN stats in fp32.  Host pre-transposes x, pre-casts
weights to bf16, and re-transposes the output.
"""

import numpy as np
import ml_dtypes

import concourse.bass as bass
from concourse import bacc
import concourse.mybir as mybir
import concourse.tile as tile
from concourse.bass import ts, ds
from concourse.bass_utils import run_bass_kernel_spmd

F32 = mybir.dt.float32
BF16 = mybir.dt.bfloat16
AF = mybir.ActivationFunctionType
ALU = mybir.AluOpType

B = 8          # batch == number of cores
N = 1024       # sequence length
D = 768        # model dim
H = 12         # heads
DH = 64        # head dim
F = 3072       # mlp dim
P = 128        # partitions
NT = N // P    # 8 seq tiles
DT = D // P    # 6 dim tiles
FT = F // P    # 24 mlp tiles
NCH = 512      # psum free-dim chunk
NC = N // NCH  # 2 chunks
HP = H // 2    # 6 head pairs
SCALE = DH ** -0.5

MM_SB = BF16
MM_NP = ml_dtypes.bfloat16


def _patch_act_tables():
    """Put the ln+exp table set first so the act-table-load pass picks one
    set for every activation in this kernel (Exp/Ln/Identity/Copy all live
    in natural_log_exp_and_others) instead of thrashing 2.7us reloads."""
    import concourse.hw_specs as _hws
    orig = _hws.get_activation_tables

    def pinned(arch):
        tabs = orig(arch)
        key = "natural_log_exp_and_others"
        if key not in tabs:
            return tabs
        # Preserve dict order (act_func_set_id is positional); empty every
        # other set so the load pass can only choose `key` for our funcs.
        return {k: (v if k == key else set()) for k, v in tabs.items()}

    bacc.get_activation_tables = pinned


def build_program():
    _patch_act_tables()
    nc = bacc.Bacc("TRN2", target_bir_lowering=False)

    xt = nc.dram_tensor("xt", [D, N], F32, kind="ExternalInput").ap()
    wq = nc.dram_tensor("wq", [D, D], MM_SB, kind="ExternalInput").ap()
    wk = nc.dram_tensor("wk", [D, D], MM_SB, kind="ExternalInput").ap()
    wv = nc.dram_tensor("wv", [D, D], MM_SB, kind="ExternalInput").ap()
    wo = nc.dram_tensor("wo", [D, D], MM_SB, kind="ExternalInput").ap()
    w1 = nc.dram_tensor("w1", [D, F], MM_SB, kind="ExternalInput").ap()
    w2 = nc.dram_tensor("w2", [F, D], MM_SB, kind="ExternalInput").ap()
    bo = nc.dram_tensor("bo", [D], F32, kind="ExternalInput").ap()
    b1 = nc.dram_tensor("b1", [F], F32, kind="ExternalInput").ap()
    b2 = nc.dram_tensor("b2", [D], F32, kind="ExternalInput").ap()
    ln1w = nc.dram_tensor("ln1w", [D], F32, kind="ExternalInput").ap()
    ln1b = nc.dram_tensor("ln1b", [D], F32, kind="ExternalInput").ap()
    ln2w = nc.dram_tensor("ln2w", [D], F32, kind="ExternalInput").ap()
    ln2b = nc.dram_tensor("ln2b", [D], F32, kind="ExternalInput").ap()
    outt = nc.dram_tensor("outt", [D, N], F32, kind="ExternalOutput").ap()

    with tile.TileContext(nc) as tc:
        _emit(nc, tc, xt, wq, wk, wv, wo, w1, w2, bo, b1, b2,
              ln1w, ln1b, ln2w, ln2b, outt)
    nc.compile()
    return nc


def _tiles(pool, n, shape, dt, tag):
    return [
        pool.tile(shape, dt, tag=f"{tag}{i}", name=f"{tag}{i}") for i in range(n)
    ]


def _emit(nc, tc, xt, wq, wk, wv, wo, w1, w2, bo, b1, b2,
          ln1w, ln1b, ln2w, ln2b, outt):
    with (
        tc.tile_pool(name="consts", bufs=1) as consts,
        tc.tile_pool(name="rows", bufs=2) as rows,
        tc.tile_pool(name="stage", bufs=4) as stage,
        tc.tile_pool(name="resid", bufs=1) as resid_pool,
    ):
        ones_full = consts.tile([P, P], MM_SB, tag="onesfull")
        nc.gpsimd.memset(ones_full[:], 1.0)

        def col_load(vec, nt, tag):
            t = consts.tile([P, nt], F32, tag=tag)
            nc.sync.dma_start(t[:], vec.rearrange("(t p) -> p t", p=P))
            return t

        ln1w_c = col_load(ln1w, DT, "ln1wc")
        ln1b_c = col_load(ln1b, DT, "ln1bc")
        ln2w_c = col_load(ln2w, DT, "ln2wc")
        ln2b_c = col_load(ln2b, DT, "ln2bc")
        bo_c = col_load(bo, DT, "boc")
        b2_c = col_load(b2, DT, "b2c")
        b1_c = col_load(b1, FT, "b1c")

        def row_load(vec, tag):
            st = consts.tile([1, D], F32, tag=tag + "f")
            nc.sync.dma_start(st[:], vec[None, :])
            t = consts.tile([P, D], MM_SB, tag=tag)
            nc.gpsimd.memset(t[:], 0.0)
            nc.vector.tensor_copy(t[0:1, :], st[:])
            return t

        ln1w_r = row_load(ln1w, "ln1wr")
        ln2w_r = row_load(ln2w, "ln2wr")

        # fp32 residual stream, updated in place phase to phase
        xts = _tiles(resid_pool, DT, [P, N], F32, "xt")
        for j in range(DT):
            nc.sync.dma_start(xts[j][:], xt[ts(j, P), :])

        def layernorm(lnps, src_tiles, src_mm_tiles, w_col, b_col, w_row,
                      out_tiles):
            """out = w * (src - mean_over_dim(src)) + b (transposed layout).

            All matmuls full 128x128 mode: sums via all-ones stationary
            (replicated rows), broadcast via w-in-row-0 stationary against a
            negmu tile with zeroed rows 1..127.
            """
            negmu = rows.tile([P, N], MM_SB, tag="negmu")
            nc.gpsimd.memset(negmu[:], 0.0)
            for ch in range(NC):
                sps = lnps.tile([P, NCH], F32, tag="lns", name="lnsums")
                for k in range(DT):
                    nc.tensor.matmul(
                        sps[:], ones_full[:], src_mm_tiles[k][:, ts(ch, NCH)],
                        start=(k == 0), stop=(k == DT - 1))
                nc.scalar.activation(negmu[0:1, ts(ch, NCH)], sps[0:1, :],
                                     AF.Copy, scale=-1.0 / D)
            for j in range(DT):
                for ch in range(NC):
                    bps = lnps.tile([P, NCH], F32, tag="lnb", name="lnbcast")
                    nc.tensor.matmul(bps[:], w_row[:, ts(j, P)],
                                     negmu[:, ts(ch, NCH)],
                                     start=True, stop=True)
                    t1 = stage.tile([P, NCH], F32, tag="st")
                    nc.vector.tensor_scalar(
                        out=t1[:], in0=src_tiles[j][:, ts(ch, NCH)],
                        scalar1=w_col[:, j:j + 1], scalar2=b_col[:, j:j + 1],
                        op0=ALU.mult, op1=ALU.add)
                    nc.vector.tensor_tensor(
                        out=out_tiles[j][:, ts(ch, NCH)], in0=t1[:],
                        in1=bps[:], op=ALU.add)

        # ============ LN1 + QKV + attention (pipelined per head pair) ====
        with (
            tc.tile_pool(name="ln1", bufs=1) as ln1_pool,
            tc.tile_pool(name="wqk", bufs=2) as wqk_pool,
            tc.tile_pool(name="wvp", bufs=1) as wv_pool,
            tc.tile_pool(name="qkv", bufs=1) as qkv_pool,
            tc.tile_pool(name="aot", bufs=1) as aot_pool,
            tc.tile_pool(name="exp", bufs=1) as exp_pool,
        ):
            hbf = _tiles(ln1_pool, DT, [P, N], MM_SB, "h")
            with (
                tc.tile_pool(name="xbfp", bufs=1) as xbf_pool,
                tc.tile_pool(name="lnps1", bufs=1, space="PSUM") as lnps1,
            ):
                xbf = _tiles(xbf_pool, DT, [P, N], MM_SB, "xb")
                for j in range(DT):
                    nc.vector.tensor_copy(xbf[j][:], xts[j][:])
                layernorm(lnps1, xts, xbf, ln1w_c, ln1b_c, ln1w_r, hbf)

            wv_sb = _tiles(wv_pool, DT, [P, D], MM_SB, "wv")
            for m in range(DT):
                nc.sync.dma_start(wv_sb[m][:], wv[ts(m, P), :])

            vbf = _tiles(qkv_pool, NT, [P, H * P], MM_SB, "v")
            aot = _tiles(aot_pool, DT, [P, N], MM_SB, "ao")

            with (
                tc.tile_pool(name="qkps", bufs=2, space="PSUM") as qkps,
                tc.tile_pool(name="dotps", bufs=2, space="PSUM") as dotps,
                tc.tile_pool(name="avps", bufs=2, space="PSUM") as avps,
            ):
                # ---- V for all heads, augmented layout ----
                # head 2t   -> vbf cols [256t,    256t+64) = v, then 64 ones
                # head 2t+1 -> vbf cols [256t+192, 256t+256) = v, ones before
                # so out2 rows and replicated-sum rows alternate alignment.
                for i in range(NT):
                    for t6 in range(HP):
                        nc.gpsimd.memset(vbf[i][:, ds(t6 * 2 * P + DH, P)], 1.0)
                    for c0, cw in ((0, NCH), (NCH, D - NCH)):
                        vps = qkps.tile([P, NCH], F32, tag="qk", name="vps")
                        for k in range(DT):
                            nc.tensor.matmul(
                                vps[:, :cw], hbf[k][:, ts(i, P)],
                                wv_sb[k][:, ds(c0, cw)],
                                start=(k == 0), stop=(k == DT - 1))
                        np_ = cw // P  # head pairs in this chunk
                        src = vps[:, :cw].rearrange("p (t x) -> p t x", x=P)
                        dst = vbf[i][:, ds(c0 * 2, np_ * 2 * P)].rearrange(
                            "p (t x) -> p t x", x=2 * P)
                        nc.vector.tensor_copy(dst[:, :, 0:DH],
                                              src[:, :, 0:DH])
                        nc.vector.tensor_copy(dst[:, :, 3 * DH:4 * DH],
                                              src[:, :, DH:2 * DH])

                def attnv_block(t, ch, head, et):
                    """One (chunk, head) block of attn@V for pair t."""
                    o0, o1 = (0, DH) if head == 0 else (DH, P)
                    s0, s1 = (DH, P) if head == 0 else (0, DH)
                    av = avps.tile([P, NCH], F32, tag="av", name="avps")
                    for ki in range(NT):
                        nc.tensor.matmul(
                            av[:], vbf[ki][:, ds((2 * t + head) * P, P)],
                            et[ki][:, ts(ch, NCH)],
                            start=(ki == 0), stop=(ki == NT - 1))
                    lns = stage.tile([P, NCH], F32, tag="st")
                    nc.scalar.activation(lns[s0:s1, :], av[s0:s1, :], AF.Ln)
                    rec = stage.tile([P, NCH], F32, tag="st")
                    nc.scalar.activation(rec[s0:s1, :], lns[s0:s1, :],
                                         AF.Exp, scale=-1.0)
                    reca = stage.tile([P, NCH], F32, tag="st")
                    nc.sync.dma_start(reca[o0:o1, :], rec[s0:s1, :])
                    nc.vector.tensor_tensor(
                        out=aot[t][o0:o1, ts(ch, NCH)],
                        in0=av[o0:o1, :], in1=reca[o0:o1, :], op=ALU.mult)

                def attnv_blocks(prev):
                    t, eta, etb = prev
                    for ch in range(NC):
                        for head, et in ((0, eta), (1, etb)):
                            yield (t, ch, head, et)

                # ---- per pair: q/k proj -> dots+exp; attnv lags one pair
                prev = None
                for t in range(HP):
                    wq_sb = wqk_pool.tile([P, DT, P], MM_SB, tag="wq",
                                          name=f"wq{t}")
                    nc.sync.dma_start(
                        wq_sb[:],
                        wq[:, ts(t, P)].rearrange("(t p) m -> p t m", p=P))
                    wk_sb = wqk_pool.tile([P, DT, P], MM_SB, tag="wk",
                                          name=f"wk{t}")
                    nc.sync.dma_start(
                        wk_sb[:],
                        wk[:, ts(t, P)].rearrange("(t p) m -> p t m", p=P))

                    qbt = qkv_pool.tile([P, N], MM_SB, tag=f"q{t % 2}",
                                        name=f"q{t}")
                    kza = qkv_pool.tile([P, N], MM_SB, tag=f"kza{t % 2}",
                                        name=f"kza{t}")
                    kzb = qkv_pool.tile([P, N], MM_SB, tag=f"kzb{t % 2}",
                                        name=f"kzb{t}")
                    nc.gpsimd.memset(kza[DH:P, :], 0.0)
                    nc.gpsimd.memset(kzb[0:DH, :], 0.0)
                    for ch in range(NC):
                        qps = qkps.tile([P, NCH], F32, tag="qk", name="qps")
                        for k in range(DT):
                            nc.tensor.matmul(
                                qps[:], wq_sb[:, k, :],
                                hbf[k][:, ts(ch, NCH)],
                                start=(k == 0), stop=(k == DT - 1))
                        nc.vector.tensor_copy(qbt[:, ts(ch, NCH)], qps[:])
                        kps = qkps.tile([P, NCH], F32, tag="qk", name="kps")
                        for k in range(DT):
                            nc.tensor.matmul(
                                kps[:], wk_sb[:, k, :],
                                hbf[k][:, ts(ch, NCH)],
                                start=(k == 0), stop=(k == DT - 1))
                        nc.vector.tensor_copy(kza[0:DH, ts(ch, NCH)],
                                              kps[0:DH, :])
                        nc.vector.tensor_copy(kzb[DH:P, ts(ch, NCH)],
                                              kps[DH:P, :])

                    # dots (zero-padded K=128, full mode) + exp
                    eta = [exp_pool.tile([P, N], MM_SB, tag="et", bufs=28,
                                         name=f"ea{t}_{i}")
                           for i in range(NT)]
                    etb = [exp_pool.tile([P, N], MM_SB, tag="et", bufs=28,
                                         name=f"eb{t}_{i}")
                           for i in range(NT)]
                    blocks = iter(attnv_blocks(prev)) if prev else iter(())
                    for mi in range(NT):
                        da = dotps.tile([P, N], F32, tag="dot", name="dpsa")
                        db = dotps.tile([P, N], F32, tag="dot", name="dpsb")
                        for ch in range(NC):
                            nc.tensor.matmul(
                                da[:, ts(ch, NCH)], kza[:, ts(mi, P)],
                                qbt[:, ts(ch, NCH)], start=True, stop=True)
                            nc.tensor.matmul(
                                db[:, ts(ch, NCH)], kzb[:, ts(mi, P)],
                                qbt[:, ts(ch, NCH)], start=True, stop=True)
                        nc.scalar.activation(eta[mi][:], da[:], AF.Exp,
                                             scale=SCALE)
                        nc.scalar.activation(etb[mi][:], db[:], AF.Exp,
                                             scale=SCALE)
                        if mi % 2 == 1:
                            b = next(blocks, None)
                            if b is not None:
                                attnv_block(*b)
                    prev = (t, eta, etb)
                for b in attnv_blocks(prev):
                    attnv_block(*b)

                # ---- Wo + residual ----
                for m in range(DT):
                    wo_sb = wqk_pool.tile([P, DT, P], MM_SB, tag="wq",
                                          name=f"wo{m}")
                    nc.sync.dma_start(
                        wo_sb[:],
                        wo[:, ts(m, P)].rearrange("(t p) m -> p t m", p=P))
                    pss = [avps.tile([P, NCH], F32, tag="av", name="wops")
                           for _ in range(NC)]
                    for k in range(DT):
                        for ch in range(NC):
                            nc.tensor.matmul(
                                pss[ch][:], wo_sb[:, k, :],
                                aot[k][:, ts(ch, NCH)],
                                start=(k == 0), stop=(k == DT - 1))
                    for ch in range(NC):
                        nc.vector.scalar_tensor_tensor(
                            out=xts[m][:, ts(ch, NCH)], in0=pss[ch][:],
                            scalar=bo_c[:, m:m + 1],
                            in1=xts[m][:, ts(ch, NCH)],
                            op0=ALU.add, op1=ALU.add)

        # ============ LN2 + FFN ============
        with (
            tc.tile_pool(name="ln2", bufs=1) as ln2_pool,
            tc.tile_pool(name="wff", bufs=1) as wff_pool,
            tc.tile_pool(name="ff1", bufs=1) as ff1_pool,
            tc.tile_pool(name="mm", bufs=6, space="PSUM") as mmps,
        ):
            # FFN weight slab DMAs issued first: overlap with LN2 compute
            w1_sb = _tiles(wff_pool, FT, [P, DT, P], MM_SB, "w1")
            for mf in range(FT):
                nc.sync.dma_start(
                    w1_sb[mf][:],
                    w1[:, ts(mf, P)].rearrange("(t p) m -> p t m", p=P))
            w2_sb = _tiles(wff_pool, DT, [P, FT, P], MM_SB, "w2")
            for m in range(DT):
                nc.sync.dma_start(
                    w2_sb[m][:],
                    w2[:, ts(m, P)].rearrange("(t p) m -> p t m", p=P))

            h2bf = _tiles(ln2_pool, DT, [P, N], MM_SB, "h2")
            with (
                tc.tile_pool(name="xmbfp", bufs=1) as xmbf_pool,
                tc.tile_pool(name="lnps2", bufs=1, space="PSUM") as lnps2,
            ):
                xmbf = _tiles(xmbf_pool, DT, [P, N], MM_SB, "xmb")
                for j in range(DT):
                    nc.vector.tensor_copy(xmbf[j][:], xts[j][:])
                layernorm(lnps2, xts, xmbf, ln2w_c, ln2b_c, ln2w_r, h2bf)

            ff1 = ff1_pool.tile([P, FT, NCH], MM_SB, tag="ff1", name="ff1")
            for ch in range(NC):
                for mf in range(FT):
                    ps = mmps.tile([P, NCH], F32, tag="mm", name="f1ps")
                    for k in range(DT):
                        nc.tensor.matmul(
                            ps[:], w1_sb[mf][:, k, :], h2bf[k][:, ts(ch, NCH)],
                            start=(k == 0), stop=(k == DT - 1))
                    nc.vector.tensor_scalar(
                        out=ff1[:, mf, :], in0=ps[:],
                        scalar1=b1_c[:, mf:mf + 1], scalar2=0.0,
                        op0=ALU.add, op1=ALU.max)
                for m in range(DT):
                    ps = mmps.tile([P, NCH], F32, tag="mm", name="f2ps")
                    for kf in range(FT):
                        nc.tensor.matmul(
                            ps[:], w2_sb[m][:, kf, :], ff1[:, kf, :],
                            start=(kf == 0), stop=(kf == FT - 1))
                    ot = stage.tile([P, NCH], F32, tag="st")
                    nc.vector.scalar_tensor_tensor(
                        out=ot[:], in0=ps[:], scalar=b2_c[:, m:m + 1],
                        in1=xts[m][:, ts(ch, NCH)],
                        op0=ALU.add, op1=ALU.add)
                    nc.sync.dma_start(outt[ts(m, P), ts(ch, NCH)], ot[:])


_CACHED = None


def _get_program():
    global _CACHED
    if _CACHED is None:
        _CACHED = build_program()
    return _CACHED


def prepare_in_maps(inputs):
    x = np.asarray(inputs["x"], dtype=np.float32)
    wcast = lambda a: np.ascontiguousarray(np.asarray(a, np.float32)).astype(MM_NP)
    f32c = lambda a: np.ascontiguousarray(np.asarray(a, np.float32))
    shared = {
        "wq": wcast(inputs["Wq"]), "wk": wcast(inputs["Wk"]),
        "wv": wcast(inputs["Wv"]), "wo": wcast(inputs["Wo"]),
        "w1": wcast(inputs["W1"]), "w2": wcast(inputs["W2"]),
        "bo": f32c(inputs["bo"]), "b1": f32c(inputs["b1"]),
        "b2": f32c(inputs["b2"]),
        "ln1w": f32c(inputs["ln1_w"]), "ln1b": f32c(inputs["ln1_b"]),
        "ln2w": f32c(inputs["ln2_w"]), "ln2b": f32c(inputs["ln2_b"]),
    }
    in_maps = []
    for i in range(B):
        m = dict(shared)
        m["xt"] = np.ascontiguousarray(x[i].T)  # [D, N]
        in_maps.append(m)
    return in_maps


def kernel(**inputs):
    nc = _get_program()
    in_maps = prepare_in_maps(inputs)
    res = run_bass_kernel_spmd(nc, in_maps, list(range(B)))
    out = np.stack([np.ascontiguousarray(r["outt"].T) for r in res.results])
    return out.astype(np.float32)


# revision 20
# speedup vs baseline: 1.2850x; 1.0195x over previous
"""ViT block kernel for Trainium2, data-parallel over batch across 8 cores.

Per-core program (sequence 1024, dim 768, 12 heads, mlp 3072), transposed
[feature, seq] layout on device end-to-end:

  LN1   : column sums via ones-matmul + rank-1 (w x -mu/D) broadcast matmul
  QKV   : weights stationary as M-slabs; V first, then per head-pair m-slab
          so attention pipelines with QKV and PE stays HAM-warm
  attn  : per head pair (2 heads share a 128-row q/k tile):
          - dots^T   = K @ Q^T, K=64 row-packed (tiles T0/T8), 2-bank PSUM
          - exp      = one ACT op per [128, 1024] tile (scale folded)
          - attn@V   = M=64 col-packed (2 heads in one [128,512] PSUM tile)
          - sums     = ones[128,64] stationary -> 64 replicated sum rows per
                       head, same packing; gives the broadcast for free
          - divide   = one DVE reciprocal + one DVE multiply per chunk
  Wo    : + residual accumulated in place into the fp32 x^T tiles
  LN2   : same as LN1
  FFN   : relu(x@W1+b1)@W2 + b2, weights streamed as M-slabs
  out   : transposed output, un-transposed on host

Matmul operands bf16 (fp32 PSUM accumulation); residual stream, softmax
sums and reciprocals, LN stats in fp32.  Host pre-transposes x, pre-casts
weights to bf16, and re-transposes the output.
"""

import numpy as np
import ml_dtypes

import concourse.bass as bass
from concourse import bacc
import concourse.mybir as mybir
import concourse.tile as tile
from concourse.bass import ts, ds
from concourse.bass_utils import run_bass_kernel_spmd

F32 = mybir.dt.float32
BF16 = mybir.dt.bfloat16
AF = mybir.ActivationFunctionType
ALU = mybir.AluOpType

B = 8          # batch == number of cores
N = 1024       # sequence length
D = 768        # model dim
H = 12         # heads
DH = 64        # head dim
F = 3072       # mlp dim
P = 128        # partitions
NT = N // P    # 8 seq tiles
DT = D // P    # 6 dim tiles
FT = F // P    # 24 mlp tiles
NCH = 512      # psum free-dim chunk
NC = N // NCH  # 2 chunks
HP = H // 2    # 6 head pairs
SCALE = DH ** -0.5

MM_SB = BF16
MM_NP = ml_dtypes.bfloat16


def _patch_act_tables():
    """Put the ln+exp table set first so the act-table-load pass picks one
    set for every activation in this kernel (Exp/Ln/Identity/Copy all live
    in natural_log_exp_and_others) instead of thrashing 2.7us reloads."""
    import concourse.hw_specs as _hws
    orig = _hws.get_activation_tables

    def pinned(arch):
        tabs = orig(arch)
        key = "natural_log_exp_and_others"
        if key not in tabs:
            return tabs
        # Preserve dict order (act_func_set_id is positional); empty every
        # other set so the load pass can only choose `key` for our funcs.
        return {k: (v if k == key else set()) for k, v in tabs.items()}

    bacc.get_activation_tables = pinned


def build_program():
    _patch_act_tables()
    nc = bacc.Bacc("TRN2", target_bir_lowering=False)

    xt = nc.dram_tensor("xt", [D, N], F32, kind="ExternalInput").ap()
    wq = nc.dram_tensor("wq", [D, D], MM_SB, kind="ExternalInput").ap()
    wk = nc.dram_tensor("wk", [D, D], MM_SB, kind="ExternalInput").ap()
    wv = nc.dram_tensor("wv", [D, D], MM_SB, kind="ExternalInput").ap()
    wo = nc.dram_tensor("wo", [D, D], MM_SB, kind="ExternalInput").ap()
    w1 = nc.dram_tensor("w1", [D, F], MM_SB, kind="ExternalInput").ap()
    w2 = nc.dram_tensor("w2", [F, D], MM_SB, kind="ExternalInput").ap()
    bo = nc.dram_tensor("bo", [D], F32, kind="ExternalInput").ap()
    b1 = nc.dram_tensor("b1", [F], F32, kind="ExternalInput").ap()
    b2 = nc.dram_tensor("b2", [D], F32, kind="ExternalInput").ap()
    ln1w = nc.dram_tensor("ln1w", [D], F32, kind="ExternalInput").ap()
    ln1b = nc.dram_tensor("ln1b", [D], F32, kind="ExternalInput").ap()
    ln2w = nc.dram_tensor("ln2w", [D], F32, kind="ExternalInput").ap()
    ln2b = nc.dram_tensor("ln2b", [D], F32, kind="ExternalInput").ap()
    outt = nc.dram_tensor("outt", [D, N], F32, kind="ExternalOutput").ap()

    with tile.TileContext(nc) as tc:
        _emit(nc, tc, xt, wq, wk, wv, wo, w1, w2, bo, b1, b2,
              ln1w, ln1b, ln2w, ln2b, outt)
    nc.compile()
    return nc


def _tiles(pool, n, shape, dt, tag):
    return [
        pool.tile(shape, dt, tag=f"{tag}{i}", name=f"{tag}{i}") for i in range(n)
    ]


def _emit(nc, tc, xt, wq, wk, wv, wo, w1, w2, bo, b1, b2,
          ln1w, ln1b, ln2w, ln2b, outt):
    with (
        tc.tile_pool(name="consts", bufs=1) as consts,
        tc.tile_pool(name="rows", bufs=2) as rows,
        tc.tile_pool(name="stage", bufs=4) as stage,
        tc.tile_pool(name="resid", bufs=1) as resid_pool,
    ):
        ones_full = consts.tile([P, P], MM_SB, tag="onesfull")
        nc.gpsimd.memset(ones_full[:], 1.0)

        def col_load(vec, nt, tag):
            t = consts.tile([P, nt], F32, tag=tag)
            nc.sync.dma_start(t[:], vec.rearrange("(t p) -> p t", p=P))
            return t

        ln1w_c = col_load(ln1w, DT, "ln1wc")
        ln1b_c = col_load(ln1b, DT, "ln1bc")
        ln2w_c = col_load(ln2w, DT, "ln2wc")
        ln2b_c = col_load(ln2b, DT, "ln2bc")
        bo_c = col_load(bo, DT, "boc")
        b2_c = col_load(b2, DT, "b2c")
        b1_c = col_load(b1, FT, "b1c")

        def row_load(vec, tag):
            st = consts.tile([1, D], F32, tag=tag + "f")
            nc.sync.dma_start(st[:], vec[None, :])
            t = consts.tile([P, D], MM_SB, tag=tag)
            nc.gpsimd.memset(t[:], 0.0)
            nc.vector.tensor_copy(t[0:1, :], st[:])
            return t

        ln1w_r = row_load(ln1w, "ln1wr")
        ln2w_r = row_load(ln2w, "ln2wr")

        # fp32 residual stream, updated in place phase to phase
        xts = _tiles(resid_pool, DT, [P, N], F32, "xt")
        for j in range(DT):
            nc.sync.dma_start(xts[j][:], xt[ts(j, P), :])

        def layernorm(lnps, src_tiles, src_mm_tiles, w_col, b_col, w_row,
                      out_tiles):
            """out = w * (src - mean_over_dim(src)) + b (transposed layout).

            All matmuls full 128x128 mode: sums via all-ones stationary
            (replicated rows), broadcast via w-in-row-0 stationary against a
            negmu tile with zeroed rows 1..127.
            """
            negmu = rows.tile([P, N], MM_SB, tag="negmu")
            nc.gpsimd.memset(negmu[:], 0.0)
            for ch in range(NC):
                sps = lnps.tile([P, NCH], F32, tag="lns", name="lnsums")
                for k in range(DT):
                    nc.tensor.matmul(
                        sps[:], ones_full[:], src_mm_tiles[k][:, ts(ch, NCH)],
                        start=(k == 0), stop=(k == DT - 1))
                nc.scalar.activation(negmu[0:1, ts(ch, NCH)], sps[0:1, :],
                                     AF.Copy, scale=-1.0 / D)
            for j in range(DT):
                for ch in range(NC):
                    bps = lnps.tile([P, NCH], F32, tag="lnb", name="lnbcast")
                    nc.tensor.matmul(bps[:], w_row[:, ts(j, P)],
                                     negmu[:, ts(ch, NCH)],
                                     start=True, stop=True)
                    t1 = stage.tile([P, NCH], F32, tag="st")
                    nc.vector.tensor_scalar(
                        out=t1[:], in0=src_tiles[j][:, ts(ch, NCH)],
                        scalar1=w_col[:, j:j + 1], scalar2=b_col[:, j:j + 1],
                        op0=ALU.mult, op1=ALU.add)
                    nc.vector.tensor_tensor(
                        out=out_tiles[j][:, ts(ch, NCH)], in0=t1[:],
                        in1=bps[:], op=ALU.add)

        # ============ LN1 + QKV + attention (pipelined per head pair) ====
        with (
            tc.tile_pool(name="ln1", bufs=1) as ln1_pool,
            tc.tile_pool(name="wqk", bufs=2) as wqk_pool,
            tc.tile_pool(name="wvp", bufs=1) as wv_pool,
            tc.tile_pool(name="qkv", bufs=1) as qkv_pool,
            tc.tile_pool(name="aot", bufs=1) as aot_pool,
            tc.tile_pool(name="exp", bufs=1) as exp_pool,
        ):
            hbf = _tiles(ln1_pool, DT, [P, N], MM_SB, "h")
            with (
                tc.tile_pool(name="xbfp", bufs=1) as xbf_pool,
                tc.tile_pool(name="lnps1", bufs=1, space="PSUM") as lnps1,
            ):
                xbf = _tiles(xbf_pool, DT, [P, N], MM_SB, "xb")
                for j in range(DT):
                    nc.vector.tensor_copy(xbf[j][:], xts[j][:])
                layernorm(lnps1, xts, xbf, ln1w_c, ln1b_c, ln1w_r, hbf)

            wv_sb = _tiles(wv_pool, DT, [P, D], MM_SB, "wv")
            for m in range(DT):
                nc.sync.dma_start(wv_sb[m][:], wv[ts(m, P), :])

            vbf = _tiles(qkv_pool, NT, [P, H * P], MM_SB, "v")
            aot = _tiles(aot_pool, DT, [P, N], MM_SB, "ao")

            with (
                tc.tile_pool(name="qkps", bufs=2, space="PSUM") as qkps,
                tc.tile_pool(name="dotps", bufs=2, space="PSUM") as dotps,
                tc.tile_pool(name="avps", bufs=2, space="PSUM") as avps,
            ):
                # ---- V for all heads, augmented layout ----
                # head 2t   -> vbf cols [256t,    256t+64) = v, then 64 ones
                # head 2t+1 -> vbf cols [256t+192, 256t+256) = v, ones before
                # so out2 rows and replicated-sum rows alternate alignment.
                for i in range(NT):
                    for t6 in range(HP):
                        nc.gpsimd.memset(vbf[i][:, ds(t6 * 2 * P + DH, P)], 1.0)
                    for c0, cw in ((0, NCH), (NCH, D - NCH)):
                        vps = qkps.tile([P, NCH], F32, tag="qk", name="vps")
                        for k in range(DT):
                            nc.tensor.matmul(
                                vps[:, :cw], hbf[k][:, ts(i, P)],
                                wv_sb[k][:, ds(c0, cw)],
                                start=(k == 0), stop=(k == DT - 1))
                        np_ = cw // P  # head pairs in this chunk
                        src = vps[:, :cw].rearrange("p (t x) -> p t x", x=P)
                        dst = vbf[i][:, ds(c0 * 2, np_ * 2 * P)].rearrange(
                            "p (t x) -> p t x", x=2 * P)
                        nc.vector.tensor_copy(dst[:, :, 0:DH],
                                              src[:, :, 0:DH])
                        nc.vector.tensor_copy(dst[:, :, 3 * DH:4 * DH],
                                              src[:, :, DH:2 * DH])

                def attnv_block(t, ch, head, et):
                    """One (chunk, head) block of attn@V for pair t."""
                    o0, o1 = (0, DH) if head == 0 else (DH, P)
                    s0, s1 = (DH, P) if head == 0 else (0, DH)
                    av = avps.tile([P, NCH], F32, tag="av", name="avps")
                    for ki in range(NT):
                        nc.tensor.matmul(
                            av[:], vbf[ki][:, ds((2 * t + head) * P, P)],
                            et[ki][:, ts(ch, NCH)],
                            start=(ki == 0), stop=(ki == NT - 1))
                    lns = stage.tile([P, NCH], F32, tag="st")
                    nc.scalar.activation(lns[s0:s1, :], av[s0:s1, :], AF.Ln)
                    rec = stage.tile([P, NCH], F32, tag="st")
                    nc.scalar.activation(rec[s0:s1, :], lns[s0:s1, :],
                                         AF.Exp, scale=-1.0)
                    reca = stage.tile([P, NCH], F32, tag="st")
                    nc.sync.dma_start(reca[o0:o1, :], rec[s0:s1, :])
                    nc.vector.tensor_tensor(
                        out=aot[t][o0:o1, ts(ch, NCH)],
                        in0=av[o0:o1, :], in1=reca[o0:o1, :], op=ALU.mult)

                def attnv_blocks(prev):
                    t, eta, etb = prev
                    for ch in range(NC):
                        for head, et in ((0, eta), (1, etb)):
                            yield (t, ch, head, et)

                # ---- per pair: q/k proj -> dots+exp; attnv lags one pair
                prev = None
                for t in range(HP):
                    wq_sb = wqk_pool.tile([P, DT, P], MM_SB, tag="wq",
                                          name=f"wq{t}")
                    nc.sync.dma_start(
                        wq_sb[:],
                        wq[:, ts(t, P)].rearrange("(t p) m -> p t m", p=P))
                    wk_sb = wqk_pool.tile([P, DT, P], MM_SB, tag="wk",
                                          name=f"wk{t}")
                    nc.sync.dma_start(
                        wk_sb[:],
                        wk[:, ts(t, P)].rearrange("(t p) m -> p t m", p=P))

                    qbt = qkv_pool.tile([P, N], MM_SB, tag=f"q{t % 2}",
                                        name=f"q{t}")
                    kza = qkv_pool.tile([P, N], MM_SB, tag=f"kza{t % 2}",
                                        name=f"kza{t}")
                    kzb = qkv_pool.tile([P, N], MM_SB, tag=f"kzb{t % 2}",
                                        name=f"kzb{t}")
                    nc.gpsimd.memset(kza[DH:P, :], 0.0)
                    nc.gpsimd.memset(kzb[0:DH, :], 0.0)
                    for ch in range(NC):
                        qps = qkps.tile([P, NCH], F32, tag="qk", name="qps")
                        for k in range(DT):
                            nc.tensor.matmul(
                                qps[:], wq_sb[:, k, :],
                                hbf[k][:, ts(ch, NCH)],
                                start=(k == 0), stop=(k == DT - 1))
                        nc.vector.tensor_copy(qbt[:, ts(ch, NCH)], qps[:])
                        kps = qkps.tile([P, NCH], F32, tag="qk", name="kps")
                        for k in range(DT):
                            nc.tensor.matmul(
                                kps[:], wk_sb[:, k, :],
                                hbf[k][:, ts(ch, NCH)],
                                start=(k == 0), stop=(k == DT - 1))
                        nc.vector.tensor_copy(kza[0:DH, ts(ch, NCH)],
                                              kps[0:DH, :])
                        nc.vector.tensor_copy(kzb[DH:P, ts(ch, NCH)],
                                              kps[DH:P, :])

                    # dots (zero-padded K=128, full mode) + exp
                    eta = [exp_pool.tile([P, N], MM_SB, tag="et", bufs=28,
                                         name=f"ea{t}_{i}")
                           for i in range(NT)]
                    etb = [exp_pool.tile([P, N], MM_SB, tag="et", bufs=28,
                                         name=f"eb{t}_{i}")
                           for i in range(NT)]
                    blocks = iter(attnv_blocks(prev)) if prev else iter(())
                    for mi in range(NT):
                        da = dotps.tile([P, N], F32, tag="dot", name="dpsa")
                        db = dotps.tile([P, N], F32, tag="dot", name="dpsb")
                        for ch in range(NC):
                            nc.tensor.matmul(
                                da[:, ts(ch, NCH)], kza[:, ts(mi, P)],
                                qbt[:, ts(ch, NCH)], start=True, stop=True)
                            nc.tensor.matmul(
                                db[:, ts(ch, NCH)], kzb[:, ts(mi, P)],
                                qbt[:, ts(ch, NCH)], start=True, stop=True)
                        nc.scalar.activation(eta[mi][:], da[:], AF.Exp,
                                             scale=SCALE)
                        nc.scalar.activation(etb[mi][:], db[:], AF.Exp,
                                             scale=SCALE)
                        if mi % 2 == 1:
                            b = next(blocks, None)
                            if b is not None:
                                attnv_block(*b)
                    prev = (t, eta, etb)
                for b in attnv_blocks(prev):
                    attnv_block(*b)

                # ---- Wo + residual ----
                for m in range(DT):
                    wo_sb = wqk_pool.tile([P, DT, P], MM_SB, tag="wq",
                                          name=f"wo{m}")
                    nc.sync.dma_start(
                        wo_sb[:],
                        wo[:, ts(m, P)].rearrange("(t p) m -> p t m", p=P))
                    pss = [dotps.tile([P, NCH], F32, tag="dot", name="wops")
                           for _ in range(NC)]
                    for k in range(DT):
                        for ch in range(NC):
                            nc.tensor.matmul(
                                pss[ch][:], wo_sb[:, k, :],
                                aot[k][:, ts(ch, NCH)],
                                start=(k == 0), stop=(k == DT - 1))
                    for ch in range(NC):
                        nc.vector.scalar_tensor_tensor(
                            out=xts[m][:, ts(ch, NCH)], in0=pss[ch][:],
                            scalar=bo_c[:, m:m + 1],
                            in1=xts[m][:, ts(ch, NCH)],
                            op0=ALU.add, op1=ALU.add)

        # ============ LN2 + FFN ============
        with (
            tc.tile_pool(name="ln2", bufs=1) as ln2_pool,
            tc.tile_pool(name="wff", bufs=1) as wff_pool,
            tc.tile_pool(name="ff1", bufs=1) as ff1_pool,
            tc.tile_pool(name="mm", bufs=6, space="PSUM") as mmps,
        ):
            # FFN weight slab DMAs issued first: overlap with LN2 compute
            w1_sb = _tiles(wff_pool, FT, [P, DT, P], MM_SB, "w1")
            for mf in range(FT):
                nc.sync.dma_start(
                    w1_sb[mf][:],
                    w1[:, ts(mf, P)].rearrange("(t p) m -> p t m", p=P))
            w2_sb = _tiles(wff_pool, DT, [P, FT, P], MM_SB, "w2")
            for m in range(DT):
                nc.sync.dma_start(
                    w2_sb[m][:],
                    w2[:, ts(m, P)].rearrange("(t p) m -> p t m", p=P))

            h2bf = _tiles(ln2_pool, DT, [P, N], MM_SB, "h2")
            with (
                tc.tile_pool(name="xmbfp", bufs=1) as xmbf_pool,
                tc.tile_pool(name="lnps2", bufs=1, space="PSUM") as lnps2,
            ):
                xmbf = _tiles(xmbf_pool, DT, [P, N], MM_SB, "xmb")
                for j in range(DT):
                    nc.vector.tensor_copy(xmbf[j][:], xts[j][:])
                layernorm(lnps2, xts, xmbf, ln2w_c, ln2b_c, ln2w_r, h2bf)

            ff1 = ff1_pool.tile([P, FT, NCH], MM_SB, tag="ff1", name="ff1")
            for ch in range(NC):
                for mf in range(FT):
                    ps = mmps.tile([P, NCH], F32, tag="mm", name="f1ps")
                    for k in range(DT):
                        nc.tensor.matmul(
                            ps[:], w1_sb[mf][:, k, :], h2bf[k][:, ts(ch, NCH)],
                            start=(k == 0), stop=(k == DT - 1))
                    nc.vector.tensor_scalar(
                        out=ff1[:, mf, :], in0=ps[:],
                        scalar1=b1_c[:, mf:mf + 1], scalar2=0.0,
                        op0=ALU.add, op1=ALU.max)
                for m in range(DT):
                    ps = mmps.tile([P, NCH], F32, tag="mm", name="f2ps")
                    for kf in range(FT):
                        nc.tensor.matmul(
                            ps[:], w2_sb[m][:, kf, :], ff1[:, kf, :],
                            start=(kf == 0), stop=(kf == FT - 1))
                    ot = stage.tile([P, NCH], F32, tag="st")
                    nc.vector.scalar_tensor_tensor(
                        out=ot[:], in0=ps[:], scalar=b2_c[:, m:m + 1],
                        in1=xts[m][:, ts(ch, NCH)],
                        op0=ALU.add, op1=ALU.add)
                    nc.sync.dma_start(outt[ts(m, P), ts(ch, NCH)], ot[:])


_CACHED = None


def _get_program():
    global _CACHED
    if _CACHED is None:
        _CACHED = build_program()
    return _CACHED


def prepare_in_maps(inputs):
    x = np.asarray(inputs["x"], dtype=np.float32)
    wcast = lambda a: np.ascontiguousarray(np.asarray(a, np.float32)).astype(MM_NP)
    f32c = lambda a: np.ascontiguousarray(np.asarray(a, np.float32))
    shared = {
        "wq": wcast(inputs["Wq"]), "wk": wcast(inputs["Wk"]),
        "wv": wcast(inputs["Wv"]), "wo": wcast(inputs["Wo"]),
        "w1": wcast(inputs["W1"]), "w2": wcast(inputs["W2"]),
        "bo": f32c(inputs["bo"]), "b1": f32c(inputs["b1"]),
        "b2": f32c(inputs["b2"]),
        "ln1w": f32c(inputs["ln1_w"]), "ln1b": f32c(inputs["ln1_b"]),
        "ln2w": f32c(inputs["ln2_w"]), "ln2b": f32c(inputs["ln2_b"]),
    }
    in_maps = []
    for i in range(B):
        m = dict(shared)
        m["xt"] = np.ascontiguousarray(x[i].T)  # [D, N]
        in_maps.append(m)
    return in_maps


def kernel(**inputs):
    nc = _get_program()
    in_maps = prepare_in_maps(inputs)
    res = run_bass_kernel_spmd(nc, in_maps, list(range(B)))
    out = np.stack([np.ascontiguousarray(r["outt"].T) for r in res.results])
    return out.astype(np.float32)
